# revision 29
# baseline (speedup 1.0000x reference)
"""Trainium2 Bass kernel for nn_CombinedRotaryEmbedding.

Math: the 32 sequential Givens rotations and the learned rotation_matrix
compose into a single 64x64 matrix M (host-precomputed).  The RoPE stage
  out_top = y1*cos - y2*sin ; out_bot = y1*sin + y2*cos
is rewritten as out = u # COS + w # SIN with
  u = x @ Mbig   (rows = [Y1 | Y2] per head-pair)
  w = x @ Msw    (rows = [-Y2 | Y1])
so no cross-partition data movement is needed on-device.

Final config (measured fastest of ~12 structural variants):
  - steady state: per group of 1024 out-cols, the Scalar engine drains
    the [u|w] PSUM pair with ONE 2048-col ACTIVATE; the DVE does ONE
    fused 2048-col 2x bf16 mul against a [cos|sin] table and ONE
    1024-col add.  GpSimd/Pool is never used for tensor work — its ops
    poison DVE SBUF bandwidth ~2.5x (measured).
  - 2048-col DMA chunks (first/last 1024) instead of 8192: the first
    drain starts ~7us earlier and the final out-DMA is 4x smaller.
  - one fused const DMA ([mb | msw | cos|sin] in a single tensor).
  - out-DMA emitted one chunk late so SP never stalls x-issue behind
    the adds.
  - a ~5us burst of dummy matmuls at kernel start (while the first DMAs
    are in flight) locks the PE activity monitor into its 2.4 GHz state
    through the pipeline ramp; steady-state PE duty (~45%) can never
    trigger the switch on its own.

Sharding: sequence-parallel over 8 cores (1024 positions each).  The host
pre-transposes x to [core][128 partitions = (head%2, d_in)][b, head//2, s]
so the PE can contract over d_in with full 128-partition utilisation, and
inverse-permutes the output.
"""

import numpy as np


def _import_bass():
    try:
        import concourse.bass  # noqa: F401
    except ImportError:
        import sys

        sys.path.insert(0, "/opt/trn_rl_repo")


_import_bass()

import concourse.bass as bass  # noqa: E402
import concourse.mybir as mybir  # noqa: E402
from concourse.tile import TileContext  # noqa: E402
from concourse.vector_clock import ScopedClock  # noqa: E402

import ml_dtypes  # noqa: E402

B, S, NSTATE = 4, 8192, 1024
H, D, NUM_ROT = 16, 64, 32
NCORES = 8
S_SH = S // NCORES  # 1024 positions per core
FREE = B * (H // 2) * S_SH  # 32768 columns per core
GROUP = 1024  # out-columns per PSUM group == one (b, head-pair) s-block
# chunk sizes in groups: small head/tail chunks for pipeline ramp
CHUNK_GROUPS = [1] + [2] * 15 + [1]
assert sum(CHUNK_GROUPS) == FREE // GROUP

F32 = mybir.dt.float32
BF16 = mybir.dt.bfloat16
NP_BF16 = ml_dtypes.bfloat16


class _TileContextSplitDrain(TileContext):
    """TileContext whose final drain carries at most one sem wait per
    instruction — the walrus in this container rejects instructions
    with 2+ sync waits ("Too many sync wait commands")."""

    def _drain_and_barrier(self, tick_clock, wait_clock):
        nc = self.nc
        drain_inst = nc.sync.drain()
        wait_clock.add_sem_waits(
            drain_inst.ins, ScopedClock({None: tick_clock.global_clock})
        )
        si = drain_inst.ins.sync_info
        waits = list(si.on_wait or [])
        if len(waits) > 1:
            si.on_wait = [waits[0]]
            for w in waits[1:]:
                n = nc.sync.nop(nofuse=True, hint="drain_wait_split")
                n.ins.sync_info = type(si)(on_update=[], on_wait=[w])
        nc.all_engine_barrier()
        assert self.sems is not None
        popped = nc._tile_sem_poison_stack.pop()
        assert popped is self._sem_poison
        nc.clear_and_free_semaphores(list(self.sems.allocated().values()))
        nc.all_engine_barrier()


def _split_excess_waits(nc, limit=1):
    """Walrus here rejects instructions with >limit sync waits.  Hoist
    excess waits onto same-engine InstNoOps inserted immediately before
    the offending instruction (same engine stream => program order)."""
    n_split = 0
    for fn in nc.m.functions:
        for blk in fn.blocks:
            insts = blk.instructions
            i = 0
            while i < len(insts):
                inst = insts[i]
                si = getattr(inst, "sync_info", None)
                waits = list(si.on_wait) if (si and si.on_wait) else []
                if len(waits) > limit:
                    keep = waits[-limit:]
                    excess = waits[:-limit]
                    si.on_wait = keep
                    for j, w in enumerate(excess):
                        nop = mybir.InstNoOp(
                            name=f"{inst.name}-wsplit{j}",
                            engine=inst.engine,
                            bass_nofuse=True,
                            sync_info=mybir.SyncInfo(on_wait=[w], on_update=[]),
                        )
                        insts.insert(i, nop)
                        i += 1
                        n_split += 1
                i += 1
    return n_split


def compose_rotation(thetas: np.ndarray, rotation_matrix: np.ndarray) -> np.ndarray:
    """Fold the sequential Givens rotations + rotation_matrix into one 64x64."""
    M = np.eye(D, dtype=np.float64)
    th = thetas.astype(np.float64)
    for k in range(NUM_ROT):
        i, j = k % D, (k + 1) % D
        c, s = np.cos(th[k]), np.sin(th[k])
        mi = M[:, i] * c + M[:, j] * s
        mj = -M[:, i] * s + M[:, j] * c
        M[:, i], M[:, j] = mi, mj
    return M @ rotation_matrix.astype(np.float64)


def build_weights(thetas: np.ndarray, rotation_matrix: np.ndarray):
    """Mbig (u = [Y1|Y2]) and Msw (w = [-Y2|Y1]) as [k=128, m=128] bf16."""
    M64 = compose_rotation(thetas, rotation_matrix)
    Mev = M64[:, 0::2]  # y1 columns [64, 32]
    Mod = M64[:, 1::2]  # y2 columns
    Mbig = np.zeros((128, 128), dtype=np.float64)
    Msw = np.zeros((128, 128), dtype=np.float64)
    for hp in (0, 1):
        r = slice(hp * 64, hp * 64 + 64)
        c1 = slice(hp * 32, hp * 32 + 32)
        c2 = slice(64 + hp * 32, 64 + hp * 32 + 32)
        Mbig[r, c1] = Mev
        Mbig[r, c2] = Mod
        Msw[r, c1] = -Mod
        Msw[r, c2] = Mev
    return Mbig.astype(NP_BF16), Msw.astype(NP_BF16)


def build_tables(inv_freq: np.ndarray):
    """Per-core fused [cos|sin] tables [128, 2048], row p uses
    inv_freq[p % 32], column j (<1024) is position core_base + j.

    Args are computed in fp32 to match the reference's fp32 `pos * inv_freq`
    rounding; sin/cos mirror the reference's jax lowering when available.
    """
    invf = inv_freq.astype(np.float32)
    try:
        import jax.numpy as jnp

        pos = jnp.arange(S, dtype=jnp.float32)
        sinusoid = pos[:, None] * jnp.asarray(invf)[None, :]  # [S, 32]
        sin_all = np.asarray(jnp.sin(sinusoid))
        cos_all = np.asarray(jnp.cos(sinusoid))
    except Exception:
        args = np.arange(S, dtype=np.float32)[:, None] * invf[None, :]
        sin_all, cos_all = np.sin(args), np.cos(args)

    l = np.arange(128) % 32
    cstabs = np.empty((NCORES, 128, 2 * GROUP), dtype=NP_BF16)
    for c in range(NCORES):
        sl = slice(c * S_SH, (c + 1) * S_SH)
        cstabs[c, :, :GROUP] = cos_all[sl].T[l].astype(NP_BF16)  # [128, 1024]
        cstabs[c, :, GROUP:] = sin_all[sl].T[l].astype(NP_BF16)
    return cstabs


def shard_x(x: np.ndarray) -> np.ndarray:
    """[B,S,1024] -> [core, 128 (hp,d), FREE (b,hi,s)] contiguous bf16."""
    xr = np.ascontiguousarray(x).reshape(B, NCORES, S_SH, H // 2, 2, D)
    xt = xr.transpose(1, 4, 5, 0, 3, 2)  # (core, hp, d, b, hi, sl)
    return np.ascontiguousarray(xt).astype(NP_BF16).reshape(NCORES, 128, FREE)


def unshard_out(o: np.ndarray) -> np.ndarray:
    """[core, 128 (half,hp,l), FREE (b,hi,s)] bf16 -> [B,S,1024] f32."""
    orr = o.astype(np.float32).reshape(NCORES, 2, 2, 32, B, H // 2, S_SH)
    ot = orr.transpose(4, 0, 6, 5, 2, 1, 3)  # (b, core, sl, hi, hp, half, l)
    return np.ascontiguousarray(ot).reshape(B, S, NSTATE)


def pack_consts(Mbig, Msw, cstab):
    """One [128, 2304] bf16 tensor: [mb | msw | cos|sin table]."""
    c = np.empty((128, 256 + 2 * GROUP), dtype=NP_BF16)
    c[:, 0:128] = Mbig
    c[:, 128:256] = Msw
    c[:, 256:] = cstab
    return c


_NC_CACHE = {}


def _build_nc():
    if "nc" in _NC_CACHE:
        return _NC_CACHE["nc"]
    nc = bass.Bass(trn_type="TRN2")
    x_d = nc.dram_tensor("x", [128, FREE], BF16, kind="ExternalInput")
    consts_d = nc.dram_tensor("consts", [128, 256 + 2 * GROUP], BF16,
                              kind="ExternalInput")
    o_d = nc.dram_tensor("o", [128, FREE], BF16, kind="ExternalOutput")

    with _TileContextSplitDrain(nc) as tc:
        with tc.tile_pool(name="const", bufs=1) as cpool, \
             tc.tile_pool(name="xin", bufs=5) as xpool, \
             tc.tile_pool(name="t12", bufs=4) as tpool, \
             tc.tile_pool(name="oout", bufs=3) as opool, \
             tc.tile_pool(name="psum", bufs=2, space="PSUM") as ppool:
            ct = cpool.tile([128, 256 + 2 * GROUP], BF16, tag="consts")
            nc.sync.dma_start(out=ct, in_=consts_d.ap())
            mb = ct[:, 0:128]
            msw = ct[:, 128:256]
            cstab = ct[:, 256:256 + 2 * GROUP]

            # PE warm-up: the PE activity monitor only raises the clock
            # 1.2->2.4 GHz after ~3.4us of SUSTAINED activity, which the
            # ~45% steady-state PE duty cycle never provides.  Burn ~5us
            # of dummy matmuls into a scratch PSUM bank while the first
            # DMAs are in flight (the PE is idle then anyway).
            warm = cpool.tile([128, 512], BF16, tag="warm")
            nc.vector.memset(warm, 0.0)
            wps = ppool.tile([128, 2 * GROUP], F32, tag="ps")
            for _ in range(12):
                nc.tensor.matmul(wps[:, 0:512], lhsT=warm[:, 0:128],
                                 rhs=warm, start=True, stop=True)

            pending = []  # (out_tile, col_offset, n_cols) not yet DMA'd
            col = 0
            for ngr in CHUNK_GROUPS:
                cols = ngr * GROUP
                xt = xpool.tile([128, cols], BF16)
                nc.sync.dma_start(out=xt, in_=x_d.ap()[:, col:col + cols])
                ot = opool.tile([128, cols], BF16)
                for g2 in range(ngr):
                    xs = xt[:, g2 * GROUP:(g2 + 1) * GROUP]
                    ps = ppool.tile([128, 2 * GROUP], F32, tag="ps")
                    # u = ps[:, 0:1024], w = ps[:, 1024:2048]
                    nc.tensor.matmul(ps[:, 0:512], lhsT=mb, rhs=xs[:, 0:512],
                                     start=True, stop=True)
                    nc.tensor.matmul(ps[:, 512:1024], lhsT=mb,
                                     rhs=xs[:, 512:1024],
                                     start=True, stop=True)
                    nc.tensor.matmul(ps[:, 1024:1536], lhsT=msw,
                                     rhs=xs[:, 0:512],
                                     start=True, stop=True)
                    nc.tensor.matmul(ps[:, 1536:2048], lhsT=msw,
                                     rhs=xs[:, 512:1024],
                                     start=True, stop=True)

                    uw = tpool.tile([128, 2 * GROUP], BF16)
                    t12 = tpool.tile([128, 2 * GROUP], BF16)
                    # single 2048-col PSUM drain on the Scalar engine
                    nc.scalar.copy(out=uw, in_=ps)
                    # one fused 2048-col 2x bf16 mul ([u|w] # [cos|sin]),
                    # then the 1024-col add
                    nc.vector.tensor_mul(out=t12, in0=uw, in1=cstab)
                    ov = ot[:, g2 * GROUP:(g2 + 1) * GROUP]
                    nc.vector.tensor_add(out=ov, in0=t12[:, 0:GROUP],
                                         in1=t12[:, GROUP:2 * GROUP])
                # emit out-DMA one chunk late so SP's x-issue never
                # blocks behind the adds
                pending.append((ot, col, cols))
                if len(pending) > 1:
                    pot, pcol, pcols = pending.pop(0)
                    nc.sync.dma_start(
                        out=o_d.ap()[:, pcol:pcol + pcols], in_=pot
                    )
                col += cols
            for pot, pcol, pcols in pending:
                nc.sync.dma_start(out=o_d.ap()[:, pcol:pcol + pcols], in_=pot)
    _split_excess_waits(nc)
    _NC_CACHE["nc"] = nc
    return nc


def kernel(x, thetas, rotation_matrix, inv_freq, _trace=False):
    from concourse.bass_utils import run_bass_kernel_spmd

    x = np.asarray(x, dtype=np.float32)
    thetas = np.asarray(thetas, dtype=np.float32)
    rotation_matrix = np.asarray(rotation_matrix, dtype=np.float32)
    inv_freq = np.asarray(inv_freq, dtype=np.float32)

    Mbig, Msw = build_weights(thetas, rotation_matrix)
    cstabs = build_tables(inv_freq)
    xs = shard_x(x)

    nc = _build_nc()
    in_maps = [
        {"x": xs[c], "consts": pack_consts(Mbig, Msw, cstabs[c])}
        for c in range(NCORES)
    ]
    res = run_bass_kernel_spmd(
        nc, in_maps, core_ids=list(range(NCORES)), trace=_trace
    )
    o = np.stack([res.results[c]["o"] for c in range(NCORES)])
    out = unshard_out(o)
    if _trace:
        return out, res
    return out
